# revision 22
# baseline (speedup 1.0000x reference)
"""NSA-style block compression (sparse_attention) Trainium2 kernel.

y[b, m, h, :] = sum_{r<32} w[r] * (x[b, 16*m + r, h, :] + pe[r, :]),  M = 1023

Decomposition used on device (per core):
  - Shard: 8 cores = 4 batches x 2 sequence-halves. Each core gets a
    contiguous [8208, 512] slice of x[b] (rows = seq positions, cols = H*D)
    and produces 512 output rows; halves overlap by one output row which the
    host drops.
  - All streamed tensors ride in bf16 (host-side cast; rel-err ~2.7e-3 vs
    the 2e-2 budget): halves the HBM traffic that bounds this kernel and
    runs the PE at its native 1 cycle/row rate.
  - 63-output tiles: chunk c covers rows [1008c, 1008c+1024) as a
    [128, 8, 512] tile (row = 8p + s), which fully contains the windows of
    outputs 63c..63c+62 — so each tile needs exactly 8 banded matmuls
    U_s[p, j] = w[8p + s - 16j] and NO window-tail pass (chunks re-read a
    16-row overlap instead). A small 9th tile (rows 8064..8207, 2 passes)
    covers outputs 504..511. 8 passes per 2.78us chunk keeps the PE faster
    than the DMA stream so it tracks the data to the end.
  - The pe bias (sum_r w[r]*pe[r,:], factored out of the gather) is added
    during PSUM->bf16 evacuation on the DVE against a broadcast tile that
    rides the software DGE.
  - DMA: quarters ([128, 2, 512], 2KB lines) — s slices 0..3 on the sync
    HWDGE ring, 4..7 on scalar, so the PE's per-slice region waits track
    the descriptors in arrival order. Chunks 0..3 are pushed up front (deep
    queue = full DGE rate); chunks c+4 are pushed from compute step c so
    the late descriptors' completion semaphores (which post behind the
    per-engine packet queues) stay fresh for the end-game.
"""

import os
import sys

sys.path.insert(0, "/opt/trn_rl_repo")

import numpy as np

_B, _N, _H, _D = 4, 16384, 4, 128
_K, _S = 32, 16
_M = (_N - _K) // _S + 1          # 1023
_F = _H * _D                      # 512
_NS = 8208                        # input rows per core
_MS = 512                         # output rows per core
_NCH = 8                          # full 63-output chunks
_CSTRIDE = 1008                   # row stride between chunks (63*16)
_WCOLS = 8 * 63 + 8 + 8 + 64      # 8 U_s blocks + V0 + V1 + bias ones row

_cache = {}


def _dtypes():
    import concourse.mybir as mybir

    xname = os.environ.get("BASS_X_DTYPE", "bfloat16")
    yname = os.environ.get("BASS_Y_DTYPE", "bfloat16")
    tab = {
        "float32": mybir.dt.float32,
        "float32r": mybir.dt.float32r,
        "bfloat16": mybir.dt.bfloat16,
    }
    return tab[xname], tab[yname]


def _build():
    if "nc" in _cache:
        return _cache["nc"]

    import concourse.bass as bass
    import concourse.mybir as mybir
    import concourse.tile as tile
    from concourse import bacc

    DT, YT = _dtypes()
    f32 = mybir.dt.float32

    nc = bacc.Bacc(None, target_bir_lowering=False, debug=False)
    xs = nc.dram_tensor("xs", [_NS, _F], DT, kind="ExternalInput")
    wbufd = nc.dram_tensor("wbufd", [128, _WCOLS], DT, kind="ExternalInput")
    biasd = nc.dram_tensor("biasd", [1, _F], DT, kind="ExternalInput")
    y = nc.dram_tensor("y", [_MS, _F], YT, kind="ExternalOutput")

    with tile.TileContext(nc) as tc:
        with (
            tc.tile_pool(name="xp", bufs=1) as xp,
            tc.tile_pool(name="wp", bufs=1) as wp,
            tc.tile_pool(name="pp", bufs=6, space=bass.MemorySpace.PSUM) as pp,
            tc.tile_pool(name="pb", bufs=1, space=bass.MemorySpace.PSUM) as pb,
            tc.tile_pool(name="op", bufs=1) as op,
        ):
            engs = [nc.sync, nc.scalar]

            # wbuf leads the sync ring, the bias row the scalar ring (1KB).
            # The bias broadcast across 64 psum partitions is built by one
            # matmul against the all-ones column block of wbuf, then copied
            # to SBUF by the (otherwise idle) DVE.
            wbuf = wp.tile([128, _WCOLS], DT, tag="wbuf")
            nc.sync.dma_start(wbuf[:], wbufd.ap())
            bias_t = wp.tile([1, _F], DT, tag="biasrow")
            nc.scalar.dma_start(bias_t[:], biasd.ap())
            bias_bc = wp.tile([64, _F], f32, tag="bias")

            xcs = {}

            def push_chunk(c, pieces):
                if c < _NCH:
                    t = xp.tile([128, 8, _F], DT, tag=f"x{c}")
                    src = xs.ap()[
                        _CSTRIDE * c : _CSTRIDE * c + 1024, :
                    ].rearrange("(p s) f -> p s f", s=8)
                    w = 8 // pieces
                    for k in range(pieces):
                        lo = w * k
                        engs[(w * k) // 4].dma_start(
                            t[:, lo : lo + w, :], src[:, lo : lo + w, :]
                        )
                else:
                    # Small 9th tile: rows 8064..8191 row-major + 8192..8207.
                    t = xp.tile([128, _F], DT, tag="x8a")
                    nc.scalar.dma_start(t[:], xs.ap()[8064:8192, :])
                    tb = xp.tile([16, _F], DT, tag="x8b")
                    nc.scalar.dma_start(tb[:], xs.ap()[8192:8208, :])
                    t = (t, tb)
                xcs[c] = t

            # All x pushes are unconditional; the 8-slot DGE semaphore pool
            # paces them at the ring drain rate. Chunks 0-1 as quarters (the
            # shallow head keeps their completion semaphores fresh so the PE
            # starts early), chunks 2-7 as halves (big descriptors bridge
            # the pool pacing stalls). The small tile rides after chunk 2 so
            # its (stale-by-one-window) semaphores still post well before
            # its compute slot between tiles 6 and 7.
            for c in range(2):
                push_chunk(c, 4)
            push_chunk(2, 2)
            push_chunk(_NCH, 0)
            for c in range(3, _NCH):
                push_chunk(c, 2)

            # Bias broadcast: ones[1, 64] x bias_row[1, 512] -> psum, DVE
            # copies it to SBUF for the evacuation adds.
            bias_ps = pb.tile([64, _F], f32)
            nc.tensor.matmul(
                bias_ps[:], wbuf[0:1, 520:584], bias_t[:],
                start=True, stop=True,
            )
            nc.vector.tensor_scalar_add(bias_bc[:], bias_ps[:], 0.0)

            # Compute: 8 banded matmuls per 63-output tile; PSUM evacuated by
            # the DVE which adds the pe bias and converts to bf16. The small
            # tile runs before tile 7 (its data is long resident by then) so
            # only tile 7's evacuation trails the x stream.
            for c in list(range(_NCH - 1)) + [_NCH, _NCH - 1]:
                if c < _NCH:
                    ps = pp.tile([63, _F], f32)
                    for s in range(8):
                        nc.tensor.matmul(
                            ps[:],
                            wbuf[:, 63 * s : 63 * (s + 1)],
                            xcs[c][:, s, :],
                            start=(s == 0),
                            stop=(s == 7),
                        )
                    nr = 63
                else:
                    ps = pp.tile([8, _F], f32)
                    ta, tb = xcs[c]
                    nc.tensor.matmul(
                        ps[:], wbuf[:, 504:512], ta[:], start=True, stop=False
                    )
                    nc.tensor.matmul(
                        ps[:], wbuf[0:16, 512:520], tb[:], start=False, stop=True
                    )
                    nr = 8

                ot = op.tile([nr, _F], YT, tag=f"o{c}")
                nc.vector.tensor_add(ot[:], ps[:], bias_bc[0:nr, :])
                engs[c % 2].dma_start(
                    y.ap()[63 * c : 63 * c + nr, :], ot[:]
                )

    nc.compile()
    _cache["nc"] = nc
    return nc


def _np_dtypes():
    import concourse.mybir as mybir

    DT, YT = _dtypes()
    return mybir.dt.np(DT), mybir.dt.np(YT)


def _host_prep(weight, pe, np_dt):
    """Banded weight blocks [128, 8*63+8+8] and the pe-bias row [1, 512]."""
    w = np.asarray(weight, dtype=np.float32)
    pe = np.asarray(pe, dtype=np.float32)
    p = np.arange(128)[:, None]
    j = np.arange(63)[None, :]
    wfull = np.zeros((128, _WCOLS), dtype=np.float32)
    for s in range(8):
        idx = 8 * p + s - 16 * j
        m = (idx >= 0) & (idx < _K)
        blk = np.zeros((128, 63), dtype=np.float32)
        blk[m] = w[np.clip(idx, 0, _K - 1)[m]]
        wfull[:, 63 * s : 63 * (s + 1)] = blk
    # Small-tile blocks: V0[p, j] = w[p - 16j] (rows 8064+p), V1[q, 7] =
    # w[16+q] (rows 8192+q feed only output 511).
    j8 = np.arange(8)[None, :]
    idx = p - 16 * j8
    m = (idx >= 0) & (idx < _K)
    blk = np.zeros((128, 8), dtype=np.float32)
    blk[m] = w[np.clip(idx, 0, _K - 1)[m]]
    wfull[:, 504:512] = blk
    wfull[:16, 512 + 7] = w[16:32]
    wfull[0, 520:584] = 1.0
    bias = (w @ pe).astype(np.float32)          # [128]
    bias_row = np.tile(bias, _H)[None, :]       # [1, 512]
    return wfull.astype(np_dt), bias_row.astype(np_dt)


LAST_RESULTS = None


def kernel(x, weight, pe, stride):
    global LAST_RESULTS
    from concourse.bass_utils import run_bass_kernel_spmd

    x = np.asarray(x, dtype=np.float32)
    assert x.shape == (_B, _N, _H, _D), x.shape
    assert int(stride) == _S

    nc = _build()
    np_dt, np_yt = _np_dtypes()
    wfull, bias_row = _host_prep(weight, pe, np_dt)

    x2 = x.reshape(_B, _N, _F)
    in_maps = []
    for b in range(_B):
        for base in (0, _N - _NS):
            shard = np.ascontiguousarray(x2[b, base : base + _NS]).astype(np_dt)
            in_maps.append({"xs": shard, "wbufd": wfull, "biasd": bias_row})

    trace_cores = None
    if os.environ.get("BASS_TRACE"):
        tc_env = os.environ.get("BASS_TRACE_CORES", "0")
        trace_cores = [int(c) for c in tc_env.split(",")]
    res = run_bass_kernel_spmd(
        nc, in_maps, core_ids=list(range(8)), trace_cores=trace_cores
    )
    LAST_RESULTS = res

    out = np.empty((_B, _M, _H, _D), dtype=np.float32)
    for b in range(_B):
        y0 = res.results[2 * b]["y"].astype(np.float32).reshape(_MS, _H, _D)
        y1 = res.results[2 * b + 1]["y"].astype(np.float32).reshape(_MS, _H, _D)
        out[b, :_MS] = y0
        out[b, _MS:] = y1[1:]
    return out


# revision 30
# speedup vs baseline: 1.0998x; 1.0998x over previous
"""NSA-style block compression (sparse_attention) Trainium2 kernel.

y[b, m, h, :] = sum_{r<32} w[r] * (x[b, 16*m + r, h, :] + pe[r, :]),  M = 1023

Decomposition used on device (per core):
  - Shard: 8 cores = 4 batches x 2 sequence-halves. Each core gets a
    contiguous [8208, 512] slice of x[b] (rows = seq positions, cols = H*D)
    and produces 512 output rows; halves overlap by one output row which the
    host drops.
  - All streamed tensors ride in bf16 (host-side cast; rel-err ~2.7e-3 vs
    the 2e-2 budget): halves the HBM traffic that bounds this kernel and
    runs the PE at its native 1 cycle/row rate.
  - 63-output tiles: chunk c covers rows [1008c, 1008c+1024) as a
    [128, 8, 512] tile (row = 8p + s), which fully contains the windows of
    outputs 63c..63c+62 — so each tile needs exactly 8 banded matmuls
    U_s[p, j] = w[8p + s - 16j] and NO window-tail pass (chunks re-read a
    16-row overlap instead). A small 9th tile (rows 8064..8207, 2 passes)
    covers outputs 504..511. 8 passes per 2.78us chunk keeps the PE faster
    than the DMA stream so it tracks the data to the end.
  - The pe bias (sum_r w[r]*pe[r,:], factored out of the gather) is added
    during PSUM->bf16 evacuation on the DVE against a broadcast tile that
    rides the software DGE.
  - DMA: quarters ([128, 2, 512], 2KB lines) — s slices 0..3 on the sync
    HWDGE ring, 4..7 on scalar, so the PE's per-slice region waits track
    the descriptors in arrival order. Chunks 0..3 are pushed up front (deep
    queue = full DGE rate); chunks c+4 are pushed from compute step c so
    the late descriptors' completion semaphores (which post behind the
    per-engine packet queues) stay fresh for the end-game.
"""

import os
import sys

sys.path.insert(0, "/opt/trn_rl_repo")

import numpy as np

_B, _N, _H, _D = 4, 16384, 4, 128
_K, _S = 32, 16
_M = (_N - _K) // _S + 1          # 1023
_F = _H * _D                      # 512
_NS = 8208                        # input rows per core
_MS = 512                         # output rows per core
_NCH = 8                          # full 63-output chunks
_CSTRIDE = 1008                   # row stride between chunks (63*16)
_WCOLS = 8 * 63 + 8 + 8           # 8 U_s blocks + V0 + V1

_cache = {}


def _dtypes():
    import concourse.mybir as mybir

    xname = os.environ.get("BASS_X_DTYPE", "bfloat16")
    yname = os.environ.get("BASS_Y_DTYPE", "bfloat16")
    tab = {
        "float32": mybir.dt.float32,
        "float32r": mybir.dt.float32r,
        "bfloat16": mybir.dt.bfloat16,
    }
    return tab[xname], tab[yname]


def _build():
    if "nc" in _cache:
        return _cache["nc"]

    import concourse.bass as bass
    import concourse.mybir as mybir
    import concourse.tile as tile
    from concourse import bacc

    DT, YT = _dtypes()
    f32 = mybir.dt.float32

    nc = bacc.Bacc(None, target_bir_lowering=False, debug=False)
    xs = nc.dram_tensor("xs", [_NS, _F], DT, kind="ExternalInput")
    wbufd = nc.dram_tensor("wbufd", [128, _WCOLS], DT, kind="ExternalInput")
    biasd = nc.dram_tensor("biasd", [1, _F], f32, kind="ExternalInput")
    y = nc.dram_tensor("y", [_MS, _F], YT, kind="ExternalOutput")

    with tile.TileContext(nc) as tc:
        with (
            tc.tile_pool(name="xp", bufs=1) as xp,
            tc.tile_pool(name="wp", bufs=1) as wp,
            tc.tile_pool(name="pp", bufs=8, space=bass.MemorySpace.PSUM) as pp,
            tc.tile_pool(name="op", bufs=1) as op,
        ):
            engs = [nc.sync, nc.scalar]

            # wbuf leads the sync ring; the bias broadcast rides the (slow,
            # otherwise idle) software DGE on gpsimd.
            wbuf = wp.tile([128, _WCOLS], DT, tag="wbuf")
            nc.sync.dma_start(wbuf[:], wbufd.ap())
            bias_bc = wp.tile([64, _F], f32, tag="bias")
            nc.gpsimd.dma_start(bias_bc[:], biasd.ap().to_broadcast((64, _F)))

            xcs = {}

            def push_chunk(c, pieces):
                if c < _NCH:
                    t = xp.tile([128, 8, _F], DT, tag=f"x{c}")
                    src = xs.ap()[
                        _CSTRIDE * c : _CSTRIDE * c + 1024, :
                    ].rearrange("(p s) f -> p s f", s=8)
                    w = 8 // pieces
                    for k in range(pieces):
                        lo = w * k
                        engs[(w * k) // 4].dma_start(
                            t[:, lo : lo + w, :], src[:, lo : lo + w, :]
                        )
                else:
                    # Small 9th tile: rows 8064..8191 row-major + 8192..8207.
                    t = xp.tile([128, _F], DT, tag="x8a")
                    nc.sync.dma_start(t[:], xs.ap()[8064:8192, :])
                    tb = xp.tile([16, _F], DT, tag="x8b")
                    nc.scalar.dma_start(tb[:], xs.ap()[8192:8208, :])
                    t = (t, tb)
                xcs[c] = t

            # All x pushes are unconditional: chunks 0-1 as quarters (the
            # shallow head keeps their completion semaphores fresh so the PE
            # starts early), chunks 2-7 as halves (big descriptors bridge the
            # 8-slot DGE semaphore pool's pacing stalls), then the small
            # tile. The pool itself paces pushes at the ring drain rate.
            for c in range(2):
                push_chunk(c, 4)
            for c in range(2, _NCH):
                push_chunk(c, 2)
            push_chunk(_NCH, 0)

            # Compute: 8 banded matmuls per 63-output tile; PSUM evacuated by
            # the DVE which adds the pe bias and converts to bf16.
            for c in range(_NCH + 1):
                if c < _NCH:
                    ps = pp.tile([63, _F], f32)
                    for s in range(8):
                        nc.tensor.matmul(
                            ps[:],
                            wbuf[:, 63 * s : 63 * (s + 1)],
                            xcs[c][:, s, :],
                            start=(s == 0),
                            stop=(s == 7),
                        )
                    nr = 63
                else:
                    ps = pp.tile([8, _F], f32)
                    ta, tb = xcs[c]
                    nc.tensor.matmul(
                        ps[:], wbuf[:, 504:512], ta[:], start=True, stop=False
                    )
                    nc.tensor.matmul(
                        ps[:], wbuf[0:16, 512:520], tb[:], start=False, stop=True
                    )
                    nr = 8

                ot = op.tile([nr, _F], YT, tag=f"o{c}")
                nc.vector.tensor_add(ot[:], ps[:], bias_bc[0:nr, :])
                engs[c % 2].dma_start(
                    y.ap()[63 * c : 63 * c + nr, :], ot[:]
                )

    nc.compile()
    _cache["nc"] = nc
    return nc


def _np_dtypes():
    import concourse.mybir as mybir

    DT, YT = _dtypes()
    return mybir.dt.np(DT), mybir.dt.np(YT)


def _host_prep(weight, pe, np_dt):
    """Banded weight blocks [128, 8*63+8+8] and the pe-bias row [1, 512]."""
    w = np.asarray(weight, dtype=np.float32)
    pe = np.asarray(pe, dtype=np.float32)
    p = np.arange(128)[:, None]
    j = np.arange(63)[None, :]
    wfull = np.zeros((128, _WCOLS), dtype=np.float32)
    for s in range(8):
        idx = 8 * p + s - 16 * j
        m = (idx >= 0) & (idx < _K)
        blk = np.zeros((128, 63), dtype=np.float32)
        blk[m] = w[np.clip(idx, 0, _K - 1)[m]]
        wfull[:, 63 * s : 63 * (s + 1)] = blk
    # Small-tile blocks: V0[p, j] = w[p - 16j] (rows 8064+p), V1[q, 7] =
    # w[16+q] (rows 8192+q feed only output 511).
    j8 = np.arange(8)[None, :]
    idx = p - 16 * j8
    m = (idx >= 0) & (idx < _K)
    blk = np.zeros((128, 8), dtype=np.float32)
    blk[m] = w[np.clip(idx, 0, _K - 1)[m]]
    wfull[:, 504:512] = blk
    wfull[:16, 512 + 7] = w[16:32]
    bias = (w @ pe).astype(np.float32)          # [128]
    bias_row = np.tile(bias, _H)[None, :]       # [1, 512]
    return wfull.astype(np_dt), bias_row


LAST_RESULTS = None


def kernel(x, weight, pe, stride):
    global LAST_RESULTS
    from concourse.bass_utils import run_bass_kernel_spmd

    x = np.asarray(x, dtype=np.float32)
    assert x.shape == (_B, _N, _H, _D), x.shape
    assert int(stride) == _S

    nc = _build()
    np_dt, np_yt = _np_dtypes()
    wfull, bias_row = _host_prep(weight, pe, np_dt)

    x2 = x.reshape(_B, _N, _F)
    in_maps = []
    for b in range(_B):
        for base in (0, _N - _NS):
            shard = np.ascontiguousarray(x2[b, base : base + _NS]).astype(np_dt)
            in_maps.append({"xs": shard, "wbufd": wfull, "biasd": bias_row})

    trace_cores = None
    if os.environ.get("BASS_TRACE"):
        tc_env = os.environ.get("BASS_TRACE_CORES", "0")
        trace_cores = [int(c) for c in tc_env.split(",")]
    res = run_bass_kernel_spmd(
        nc, in_maps, core_ids=list(range(8)), trace_cores=trace_cores
    )
    LAST_RESULTS = res

    out = np.empty((_B, _M, _H, _D), dtype=np.float32)
    for b in range(_B):
        y0 = res.results[2 * b]["y"].astype(np.float32).reshape(_MS, _H, _D)
        y1 = res.results[2 * b + 1]["y"].astype(np.float32).reshape(_MS, _H, _D)
        out[b, :_MS] = y0
        out[b, _MS:] = y1[1:]
    return out
